# revision 10
# baseline (speedup 1.0000x reference)
"""Trainium2 Bass kernel for the DescriptorMatcher all-pairs problem.

Reference semantics (v1 = out1 as [N1, C], v2 = out2 as [N2, C]):
  out[n1*N2+n2, o]   = sum_c v1[n1,c] * W[o,c] * v2[n2,c] + bias[o]
  out_norm[0,n1,n2]  = || v1/(eps+|v1|) - v2/(eps+|v2|) ||
                     = sqrt(2 - 2*r1[n1]*r2[n2]*G[n1,n2])  (+O(3e-7))
  with G = v1 @ v2.T and r = 1/(eps+|v|).

Sharding: N1 split across 8 cores (128 rows each), v2 replicated.

Device program highlights:
  - Big matmuls in bf16 with mantissa splits (h+l1+l2 covers f32)
    K-stacked in pairs; per [128,1024] output three K=128 bf16 passes:
    [h;l1]@[yh;yh] + [h;l1]@[yl1;yl1] + [h;l2]@[yl2;yh]  (err ~2^-26).
    The out-head optionally runs 2-pass (drops the l2 cross terms).
  - r1/r2 norm chains in column orientation (128 lanes) with one Newton
    step to fix the ACT Sqrt table error (~7e-6 -> ~1e-7).
  - r2 row: 3-way bf16 split in columns, one PE transpose [128,24] ->
    [24,128], SBUF->SBUF cast-DMA reshape to [3,1024] bf16, then ONE
    K=3 ones-matmul sums the components while broadcasting -> r2B.
  - dist = ACT_Sqrt((G_psum * -2r1) * r2B + 2.0).
"""

import numpy as np

C = 64
N1 = 1024
N2 = 1024
NCORES = 8
SLICE = N1 // NCORES  # 128
EPS = 1e-6
H = 512  # N-half
HEAD_3PASS = False  # True: exact-ish out head (3 bf16 passes per channel)

_CACHE = {}


def _split3_np(x):
    import ml_dtypes

    bf = ml_dtypes.bfloat16
    h = x.astype(bf)
    r1 = (x - h.astype(np.float32)).astype(np.float32)
    l1 = r1.astype(bf)
    l2 = (r1 - l1.astype(np.float32)).astype(bf)
    return h, l1, l2


def _build_program():
    import concourse.bacc as bacc
    import concourse.mybir as mybir
    import concourse.tile as tile
    from concourse._compat import get_trn_type

    fp32 = mybir.dt.float32
    bf16 = mybir.dt.bfloat16
    AF = mybir.ActivationFunctionType
    OP = mybir.AluOpType
    X = mybir.AxisListType.X

    nc = bacc.Bacc(get_trn_type() or "TRN2", target_bir_lowering=False, debug=False)

    # ---- DRAM I/O (packed into 3 inputs)
    # f32e: x2t   [128, 512] f32 (first in, feeds the r2 chain)
    f32e = nc.dram_tensor("f32e", [SLICE, 512], fp32, kind="ExternalInput").ap()
    # f32f: x1t | identity | bcol   [128, 64+128+2] f32
    f32f = nc.dram_tensor(
        "f32f", [SLICE, C + 128 + 2], fp32, kind="ExternalInput"
    ).ap()
    # x1pk (bf16): x1-f32-bitcast(260) | x1h | x1l1 | x1l2   [64, 260+384]
    x1pk = nc.dram_tensor(
        "x1pk", [C, 2 * (SLICE + 2) + 3 * SLICE], bf16, kind="ExternalInput"
    ).ap()
    # rstk: x2 K-stacked bf16 rhs: [x2h;x2h] | [x2l1;x2l1] | [x2l2;x2h]
    rstk = nc.dram_tensor("rstk", [2 * C, 3 * N2], bf16, kind="ExternalInput").ap()
    out_o = nc.dram_tensor("out_o", [SLICE, N2, 2], fp32, kind="ExternalOutput").ap()
    out_n = nc.dram_tensor("out_n", [SLICE, N2], fp32, kind="ExternalOutput").ap()

    with tile.TileContext(nc) as tc:
        with (
            tc.tile_pool(name="sb", bufs=1) as sb,
            tc.tile_pool(name="ps", bufs=1, space="PSUM") as ps,
        ):
            # ================= input DMAs =================
            tf32e = sb.tile([SLICE, 512], fp32)
            nc.sync.dma_start(tf32e[:], f32e)
            tf32f = sb.tile([SLICE, C + 128 + 2], fp32)
            nc.sync.dma_start(tf32f[:], f32f)
            tx1pk = sb.tile([C, 2 * (SLICE + 2) + 3 * SLICE], bf16)
            nc.sync.dma_start(tx1pk[:], x1pk)
            trstk = sb.tile([2 * C, 3 * N2], bf16)
            nc.scalar.dma_start(trstk[:], rstk)

            x2t = tf32e[:, :]  # [128, 8*64] flat
            x1t = tf32f[:, 0:C]
            ident = tf32f[:, C : C + 128]
            bcol = tf32f[:, C + 128 : C + 128 + 2]

            # constants + ACT table prefetch (Identity) while DMAs land
            two = sb.tile([SLICE, 1], fp32)
            nc.vector.memset(two[:], 2.0)
            zerob = sb.tile([SLICE, 1], fp32)
            nc.vector.memset(zerob[:], 0.0)
            dummy = sb.tile([SLICE, 1], fp32)
            nc.scalar.activation(dummy[:], two[:], AF.Identity, bias=zerob[:],
                                 scale=1.0)
            x1f32 = tx1pk[:, 0 : 2 * (SLICE + 2)].bitcast(fp32)  # [64, 130]
            x1f = x1f32[:, 0:SLICE]
            wcs = x1f32[:, SLICE : SLICE + 2]
            o1 = 2 * (SLICE + 2)
            x1h = tx1pk[:, o1 : o1 + SLICE]
            x1l1 = tx1pk[:, o1 + SLICE : o1 + 2 * SLICE]
            x1l2 = tx1pk[:, o1 + 2 * SLICE : o1 + 3 * SLICE]
            R_hh = trstk[:, 0:N2]
            R_l1 = trstk[:, N2 : 2 * N2]
            R_mx = trstk[:, 2 * N2 : 3 * N2]

            # ================= r2 chain (col, [128,8]) =================
            x2sq = sb.tile([SLICE, 512], fp32)
            nc.vector.tensor_mul(x2sq[:], x2t, x2t)
            s2c = sb.tile([SLICE, 8], fp32)
            nc.vector.reduce_sum(
                s2c[:], x2sq[:].rearrange("p (t c) -> p t c", c=C), axis=X
            )
            a2 = sb.tile([SLICE, 8], fp32)
            nc.scalar.activation(a2[:], s2c[:], AF.Sqrt, bias=zerob[:], scale=1.0)
            i2 = sb.tile([SLICE, 8], fp32)
            nc.vector.reciprocal(i2[:], a2[:])
            t2 = sb.tile([SLICE, 8], fp32)
            nc.vector.tensor_mul(t2[:], s2c[:], i2[:])
            u2 = sb.tile([SLICE, 8], fp32)
            nc.vector.tensor_add(u2[:], a2[:], t2[:])
            n2c = sb.tile([SLICE, 8], fp32)
            nc.vector.tensor_scalar(
                out=n2c[:], in0=u2[:], scalar1=0.5, scalar2=EPS,
                op0=OP.mult, op1=OP.add,
            )
            r2c8 = sb.tile([SLICE, 8], fp32)
            nc.vector.reciprocal(r2c8[:], n2c[:])
            # 3-way bf16 split (values held in f32), packed [128, 24]
            T24 = sb.tile([SLICE, 24], fp32)
            bsc = sb.tile([SLICE, 8], bf16)
            nc.scalar.copy(bsc[:], r2c8[:])
            nc.scalar.copy(T24[:, 0:8], bsc[:])
            res1 = sb.tile([SLICE, 8], fp32)
            nc.vector.tensor_tensor(
                out=res1[:], in0=r2c8[:], in1=T24[:, 0:8], op=OP.subtract
            )
            b1c = sb.tile([SLICE, 8], bf16)
            nc.scalar.copy(b1c[:], res1[:])
            nc.scalar.copy(T24[:, 8:16], b1c[:])
            nc.vector.tensor_tensor(
                out=T24[:, 16:24], in0=res1[:], in1=T24[:, 8:16], op=OP.subtract
            )
            # transpose -> [24,128] psum -> sbuf -> cast reshape [3,1024] bf16
            ptr = ps.tile([24, 128], fp32, tag="rb")
            nc.tensor.transpose(ptr[:], T24[:], ident)
            r2t24 = sb.tile([24, 128], fp32)
            nc.vector.tensor_copy(r2t24[:], ptr[:])
            r2row3 = sb.tile([3, N2], bf16)
            nc.gpsimd.dma_start(r2row3[:], r2t24[:])  # SWDGE casts f32->bf16
            ones3 = sb.tile([3, SLICE], bf16)
            nc.vector.memset(ones3[:], 1.0)

            # ================= r1 chain (col, [128,1]) =================
            x1sq = sb.tile([SLICE, C], fp32)
            nc.vector.tensor_mul(x1sq[:], x1t, x1t)
            s1c = sb.tile([SLICE, 1], fp32)
            nc.vector.reduce_sum(s1c[:], x1sq[:], axis=X)
            a1 = sb.tile([SLICE, 1], fp32)
            nc.scalar.activation(a1[:], s1c[:], AF.Sqrt, bias=zerob[:], scale=1.0)
            i1 = sb.tile([SLICE, 1], fp32)
            nc.vector.reciprocal(i1[:], a1[:])
            t1 = sb.tile([SLICE, 1], fp32)
            nc.vector.tensor_mul(t1[:], s1c[:], i1[:])
            u1 = sb.tile([SLICE, 1], fp32)
            nc.vector.tensor_add(u1[:], a1[:], t1[:])
            n1c = sb.tile([SLICE, 1], fp32)
            nc.vector.tensor_scalar(
                out=n1c[:], in0=u1[:], scalar1=0.5, scalar2=EPS,
                op0=OP.mult, op1=OP.add,
            )
            r1c = sb.tile([SLICE, 1], fp32)
            nc.vector.reciprocal(r1c[:], n1c[:])
            r1m2 = sb.tile([SLICE, 1], fp32)
            nc.vector.tensor_scalar_mul(r1m2[:], r1c[:], -2.0)

            # ================= lhsT preps =================
            g_s1 = sb.tile([2 * C, SLICE], bf16)  # [x1h; x1l1]
            nc.vector.tensor_copy(g_s1[0:C, :], x1h)
            nc.vector.tensor_copy(g_s1[C:, :], x1l1)
            g_s2 = sb.tile([2 * C, SLICE], bf16)  # [x1h; x1l2]
            nc.vector.tensor_copy(g_s2[0:C, :], x1h)
            nc.vector.tensor_copy(g_s2[C:, :], x1l2)

            heads = []
            for o in range(2):
                lf = sb.tile([C, SLICE], fp32, tag=f"lf{o}")
                nc.vector.tensor_scalar_mul(lf[:], x1f, wcs[:, o : o + 1])
                s1t = sb.tile([2 * C, SLICE], bf16, tag=f"hs1{o}")  # [h; l1]
                nc.scalar.copy(s1t[0:C, :], lf[:])
                l1b = sb.tile([C, SLICE], bf16, tag=f"l1b{o}")
                r1f = sb.tile([C, SLICE], fp32, tag=f"r1f{o}")
                nc.vector.tensor_tensor(
                    out=r1f[:], in0=lf[:], in1=s1t[0:C, :], op=OP.subtract
                )
                nc.scalar.copy(l1b[:], r1f[:])
                nc.scalar.copy(s1t[C:, :], l1b[:])
                if HEAD_3PASS:
                    s2t = sb.tile([2 * C, SLICE], bf16, tag=f"hs2{o}")  # [h; l2]
                    nc.scalar.copy(s2t[0:C, :], s1t[0:C, :])
                    r2f = sb.tile([C, SLICE], fp32, tag=f"r2f{o}")
                    nc.vector.tensor_tensor(
                        out=r2f[:], in0=r1f[:], in1=l1b[:], op=OP.subtract
                    )
                    nc.scalar.copy(s2t[C:, :], r2f[:])
                else:
                    s2t = None
                heads.append((s1t, s2t))

            # ================= big matmuls =================
            pg = ps.tile([SLICE, N2], fp32, tag="pg")
            p0 = ps.tile([SLICE, N2], fp32, tag="p0")
            p1 = ps.tile([SLICE, N2], fp32, tag="p1")
            for j in range(2):
                sl = slice(j * H, (j + 1) * H)
                nc.tensor.matmul(pg[:, sl], g_s1[:], R_hh[:, sl],
                                 start=True, stop=False)
                nc.tensor.matmul(pg[:, sl], g_s1[:], R_l1[:, sl],
                                 start=False, stop=False)
                nc.tensor.matmul(pg[:, sl], g_s2[:], R_mx[:, sl],
                                 start=False, stop=True)

            # r2 broadcast: ONE K=3 bf16 matmul per half sums h+l1+l2
            r2B = ps.tile([SLICE, N2], fp32, tag="rb")
            for j in range(2):
                sl = slice(j * H, (j + 1) * H)
                nc.tensor.matmul(r2B[:, sl], ones3[:], r2row3[:, sl])

            for j in range(2):
                sl = slice(j * H, (j + 1) * H)
                for o, pt in ((0, p0), (1, p1)):
                    s1t, s2t = heads[o]
                    if HEAD_3PASS:
                        nc.tensor.matmul(pt[:, sl], s1t[:], R_hh[:, sl],
                                         start=True, stop=False)
                        nc.tensor.matmul(pt[:, sl], s1t[:], R_l1[:, sl],
                                         start=False, stop=False)
                        nc.tensor.matmul(pt[:, sl], s2t[:], R_mx[:, sl],
                                         start=False, stop=True)
                    else:
                        nc.tensor.matmul(pt[:, sl], s1t[:], R_hh[:, sl],
                                         start=True, stop=False)
                        nc.tensor.matmul(pt[:, sl], s1t[:], R_l1[:, sl],
                                         start=False, stop=True)

            # ================= outputs (halves, pipelined) =========
            r2Bs = sb.tile([SLICE, N2], fp32)
            d2 = sb.tile([SLICE, N2], fp32)
            dist = sb.tile([SLICE, N2], fp32)
            outsb = sb.tile([SLICE, N2, 2], fp32)
            for j in range(2):
                sl = slice(j * H, (j + 1) * H)
                nc.scalar.copy(r2Bs[:, sl], r2B[:, sl])
                nc.vector.scalar_tensor_tensor(
                    out=d2[:, sl], in0=pg[:, sl], scalar=r1m2[:],
                    in1=r2Bs[:, sl], op0=OP.mult, op1=OP.mult,
                )
                nc.scalar.activation(
                    dist[:, sl], d2[:, sl], AF.Sqrt, bias=two[:], scale=1.0
                )
                nc.sync.dma_start(out_n[:, sl], dist[:, sl])
            for j in range(2):
                sl = slice(j * H, (j + 1) * H)
                nc.vector.tensor_scalar_add(outsb[:, sl, 0], p0[:, sl], bcol[:, 0:1])
                nc.scalar.activation(
                    outsb[:, sl, 1], p1[:, sl], AF.Identity,
                    bias=bcol[:, 1:2], scale=1.0,
                )
                nc.scalar.dma_start(out_o[:, sl, :], outsb[:, sl, :])

    nc.compile()
    return nc


def _get_program():
    if "nc" not in _CACHE:
        _CACHE["nc"] = _build_program()
    return _CACHE["nc"]


def make_in_maps(out1, out2, W, bias):
    import ml_dtypes

    bf = ml_dtypes.bfloat16
    v1 = np.ascontiguousarray(out1.reshape(C, N1), dtype=np.float32)
    v2 = np.ascontiguousarray(out2.reshape(C, N2), dtype=np.float32)
    W = np.asarray(W, dtype=np.float32)
    bias = np.asarray(bias, dtype=np.float32)

    x2h, x2l1, x2l2 = _split3_np(v2)
    rstk = np.ascontiguousarray(
        np.concatenate(
            [
                np.concatenate([x2h, x2h], axis=0),
                np.concatenate([x2l1, x2l1], axis=0),
                np.concatenate([x2l2, x2h], axis=0),
            ],
            axis=1,
        )
    )

    ident = np.eye(128, dtype=np.float32)
    v2t = v2.T.reshape(8, 128, C).transpose(1, 0, 2).reshape(128, 512)
    bcolfull = np.repeat(bias[None, :], 128, axis=0)

    in_maps = []
    for k in range(NCORES):
        x1 = np.ascontiguousarray(v1[:, k * SLICE : (k + 1) * SLICE])
        h, l1, l2 = _split3_np(x1)
        f32part = np.concatenate([x1, W.T], axis=1)  # [64, 130] f32
        x1pk = np.ascontiguousarray(
            np.concatenate(
                [f32part.view(bf).reshape(C, -1), h, l1, l2], axis=1
            )
        )
        f32f = np.ascontiguousarray(
            np.concatenate([x1.T, ident, bcolfull], axis=1)
        )
        in_maps.append(
            {"f32e": v2t, "f32f": f32f, "x1pk": x1pk, "rstk": rstk}
        )
    return in_maps


def gather_results(results):
    out = np.concatenate(
        [results[k]["out_o"].reshape(SLICE * N2, 2) for k in range(NCORES)], axis=0
    )
    out_norm = np.concatenate([results[k]["out_n"] for k in range(NCORES)], axis=0)[
        None, :, :
    ]
    return out, out_norm


def kernel(out1, out2, W, bias):
    from concourse.bass_utils import run_bass_kernel_spmd

    nc = _get_program()
    in_maps = make_in_maps(
        np.asarray(out1), np.asarray(out2), np.asarray(W), np.asarray(bias)
    )
    res = run_bass_kernel_spmd(nc, in_maps, list(range(NCORES)))
    return gather_results(res.results)


# revision 11
# speedup vs baseline: 1.0977x; 1.0977x over previous
"""Trainium2 Bass kernel for the DescriptorMatcher all-pairs problem.

Reference semantics (v1 = out1 as [N1, C], v2 = out2 as [N2, C]):
  out[n1*N2+n2, o]   = sum_c v1[n1,c] * W[o,c] * v2[n2,c] + bias[o]
  out_norm[0,n1,n2]  = || v1/(eps+|v1|) - v2/(eps+|v2|) ||
                     = sqrt(2 - 2*r1[n1]*r2[n2]*G[n1,n2])  (+O(3e-7))
  with G = v1 @ v2.T and r = 1/(eps+|v|).

Sharding: N1 split across 8 cores (128 rows each), v2 replicated.

Device program highlights:
  - Big matmuls in bf16 with mantissa splits (h+l1+l2 covers f32)
    K-stacked in pairs; per [128,1024] output three K=128 bf16 passes:
    [h;l1]@[yh;yh] + [h;l1]@[yl1;yl1] + [h;l2]@[yl2;yh]  (err ~2^-26).
    The out-head optionally runs 2-pass (drops the l2 cross terms).
  - r1/r2 norm chains in column orientation (128 lanes) with one Newton
    step to fix the ACT Sqrt table error (~7e-6 -> ~1e-7).
  - r2 row: 3-way bf16 split in columns, one PE transpose [128,24] ->
    [24,128], SBUF->SBUF cast-DMA reshape to [3,1024] bf16, then ONE
    K=3 ones-matmul sums the components while broadcasting -> r2B.
  - dist = ACT_Sqrt((G_psum * -2r1) * r2B + 2.0).
"""

import numpy as np

C = 64
N1 = 1024
N2 = 1024
NCORES = 8
SLICE = N1 // NCORES  # 128
EPS = 1e-6
H = 512  # N-half
HEAD_3PASS = False  # True: exact-ish out head (3 bf16 passes per channel)

_CACHE = {}


def _split3_np(x):
    import ml_dtypes

    bf = ml_dtypes.bfloat16
    h = x.astype(bf)
    r1 = (x - h.astype(np.float32)).astype(np.float32)
    l1 = r1.astype(bf)
    l2 = (r1 - l1.astype(np.float32)).astype(bf)
    return h, l1, l2


def _build_program():
    import concourse.bacc as bacc
    import concourse.mybir as mybir
    import concourse.tile as tile
    from concourse._compat import get_trn_type

    fp32 = mybir.dt.float32
    bf16 = mybir.dt.bfloat16
    AF = mybir.ActivationFunctionType
    OP = mybir.AluOpType
    X = mybir.AxisListType.X

    nc = bacc.Bacc(get_trn_type() or "TRN2", target_bir_lowering=False, debug=False)

    # Trim the framework preamble: drop the const-AP memsets (unused; all
    # our activation biases are explicit APs) and the start all-engine
    # barrier. Body ordering is fully covered by Tile-generated semaphores,
    # and removing the barrier stops every engine from waiting ~7us for the
    # GPSIMD Q7 core to boot before even issuing input DMAs.
    _blk = nc.m.functions[0].blocks[0]
    _blk.instructions = [
        i for i in _blk.instructions
        if type(i).__name__ not in ("InstMemset", "InstDrain", "InstEventSemaphore")
    ]

    # ---- DRAM I/O (packed into 3 inputs)
    # f32e: x2t | g1-bitcast | g2-bitcast   [128, 512+64+64] f32
    # (g1 = [x1h;x1l1], g2 = [x1h;x1l2] bf16 [128,128] carried as f32 bytes)
    f32e = nc.dram_tensor("f32e", [SLICE, 512 + 128], fp32, kind="ExternalInput").ap()
    # f32f: x1t | identity | bcol   [128, 64+128+2] f32
    f32f = nc.dram_tensor(
        "f32f", [SLICE, C + 128 + 2], fp32, kind="ExternalInput"
    ).ap()
    # x1pk (bf16): x1-f32-bitcast | wc-bitcast   [64, 260]
    x1pk = nc.dram_tensor(
        "x1pk", [C, 2 * (SLICE + 2)], bf16, kind="ExternalInput"
    ).ap()
    # rstk: x2 K-stacked bf16 rhs: [x2h;x2h] | [x2l1;x2l1] | [x2l2;x2h]
    rstk = nc.dram_tensor("rstk", [2 * C, 3 * N2], bf16, kind="ExternalInput").ap()
    out_o = nc.dram_tensor("out_o", [SLICE, N2, 2], fp32, kind="ExternalOutput").ap()
    out_n = nc.dram_tensor("out_n", [SLICE, N2], fp32, kind="ExternalOutput").ap()

    with tile.TileContext(nc) as tc:
        with (
            tc.tile_pool(name="sb", bufs=1) as sb,
            tc.tile_pool(name="ps", bufs=1, space="PSUM") as ps,
        ):
            # ================= input DMAs =================
            tf32e = sb.tile([SLICE, 512 + 128], fp32)
            nc.sync.dma_start(tf32e[:], f32e)
            tx1pk = sb.tile([C, 2 * (SLICE + 2)], bf16)
            nc.sync.dma_start(tx1pk[:], x1pk)
            trstk = sb.tile([2 * C, 3 * N2], bf16)
            nc.scalar.dma_start(trstk[:], rstk)
            tf32f = sb.tile([SLICE, C + 128 + 2], fp32)
            nc.scalar.dma_start(tf32f[:], f32f)

            x2t = tf32e[:, 0:512]  # [128, 8*64] flat
            g_s1 = tf32e[:, 512:576].bitcast(bf16)  # [128, 128] [x1h;x1l1]
            g_s2 = tf32e[:, 576:640].bitcast(bf16)  # [128, 128] [x1h;x1l2]
            x1t = tf32f[:, 0:C]
            ident = tf32f[:, C : C + 128]
            bcol = tf32f[:, C + 128 : C + 128 + 2]

            # constants + ACT table prefetch (Identity) while DMAs land
            two = sb.tile([SLICE, 1], fp32)
            nc.vector.memset(two[:], 2.0)
            zerob = sb.tile([SLICE, 1], fp32)
            nc.vector.memset(zerob[:], 0.0)
            dummy = sb.tile([SLICE, 1], fp32)
            nc.scalar.activation(dummy[:], two[:], AF.Identity, bias=zerob[:],
                                 scale=1.0)
            x1f32 = tx1pk[:, 0 : 2 * (SLICE + 2)].bitcast(fp32)  # [64, 130]
            x1f = x1f32[:, 0:SLICE]
            wcs = x1f32[:, SLICE : SLICE + 2]
            R_hh = trstk[:, 0:N2]
            R_l1 = trstk[:, N2 : 2 * N2]
            R_mx = trstk[:, 2 * N2 : 3 * N2]

            # ================= r2 chain (col, [128,8]) =================
            x2sq = sb.tile([SLICE, 512], fp32)
            nc.vector.tensor_mul(x2sq[:], x2t, x2t)
            s2c = sb.tile([SLICE, 8], fp32)
            nc.vector.reduce_sum(
                s2c[:], x2sq[:].rearrange("p (t c) -> p t c", c=C), axis=X
            )
            a2 = sb.tile([SLICE, 8], fp32)
            nc.scalar.activation(a2[:], s2c[:], AF.Sqrt, bias=zerob[:], scale=1.0)
            i2 = sb.tile([SLICE, 8], fp32)
            nc.vector.reciprocal(i2[:], a2[:])
            t2 = sb.tile([SLICE, 8], fp32)
            nc.vector.tensor_mul(t2[:], s2c[:], i2[:])
            u2 = sb.tile([SLICE, 8], fp32)
            nc.vector.tensor_add(u2[:], a2[:], t2[:])
            n2c = sb.tile([SLICE, 8], fp32)
            nc.vector.tensor_scalar(
                out=n2c[:], in0=u2[:], scalar1=0.5, scalar2=EPS,
                op0=OP.mult, op1=OP.add,
            )
            r2c8 = sb.tile([SLICE, 8], fp32)
            nc.vector.reciprocal(r2c8[:], n2c[:])
            # 3-way bf16 split (values held in f32), packed [128, 24]
            T24 = sb.tile([SLICE, 24], fp32)
            bsc = sb.tile([SLICE, 8], bf16)
            nc.scalar.copy(bsc[:], r2c8[:])
            nc.scalar.copy(T24[:, 0:8], bsc[:])
            res1 = sb.tile([SLICE, 8], fp32)
            nc.vector.tensor_tensor(
                out=res1[:], in0=r2c8[:], in1=T24[:, 0:8], op=OP.subtract
            )
            b1c = sb.tile([SLICE, 8], bf16)
            nc.scalar.copy(b1c[:], res1[:])
            nc.scalar.copy(T24[:, 8:16], b1c[:])
            nc.vector.tensor_tensor(
                out=T24[:, 16:24], in0=res1[:], in1=T24[:, 8:16], op=OP.subtract
            )
            ones3 = sb.tile([3, SLICE], bf16)
            nc.vector.memset(ones3[:], 1.0)

            # ================= r1 chain (col, [128,1]) =================
            x1sq = sb.tile([SLICE, C], fp32)
            nc.vector.tensor_mul(x1sq[:], x1t, x1t)
            s1c = sb.tile([SLICE, 1], fp32)
            nc.vector.reduce_sum(s1c[:], x1sq[:], axis=X)
            a1 = sb.tile([SLICE, 1], fp32)
            nc.scalar.activation(a1[:], s1c[:], AF.Sqrt, bias=zerob[:], scale=1.0)
            i1 = sb.tile([SLICE, 1], fp32)
            nc.vector.reciprocal(i1[:], a1[:])
            t1 = sb.tile([SLICE, 1], fp32)
            nc.vector.tensor_mul(t1[:], s1c[:], i1[:])
            u1 = sb.tile([SLICE, 1], fp32)
            nc.vector.tensor_add(u1[:], a1[:], t1[:])
            n1c = sb.tile([SLICE, 1], fp32)
            nc.vector.tensor_scalar(
                out=n1c[:], in0=u1[:], scalar1=0.5, scalar2=EPS,
                op0=OP.mult, op1=OP.add,
            )
            r1c = sb.tile([SLICE, 1], fp32)
            nc.vector.reciprocal(r1c[:], n1c[:])
            r1m2 = sb.tile([SLICE, 1], fp32)
            nc.vector.tensor_scalar_mul(r1m2[:], r1c[:], -2.0)

            # ================= lhsT preps (out head) =================
            heads = []
            for o in range(2):
                lf = sb.tile([C, SLICE], fp32, tag=f"lf{o}")
                nc.vector.tensor_scalar_mul(lf[:], x1f, wcs[:, o : o + 1])
                s1t = sb.tile([2 * C, SLICE], bf16, tag=f"hs1{o}")  # [h; l1]
                nc.scalar.copy(s1t[0:C, :], lf[:])
                l1b = sb.tile([C, SLICE], bf16, tag=f"l1b{o}")
                r1f = sb.tile([C, SLICE], fp32, tag=f"r1f{o}")
                nc.vector.tensor_tensor(
                    out=r1f[:], in0=lf[:], in1=s1t[0:C, :], op=OP.subtract
                )
                nc.scalar.copy(l1b[:], r1f[:])
                nc.scalar.copy(s1t[C:, :], l1b[:])
                if HEAD_3PASS:
                    s2t = sb.tile([2 * C, SLICE], bf16, tag=f"hs2{o}")  # [h; l2]
                    nc.scalar.copy(s2t[0:C, :], s1t[0:C, :])
                    r2f = sb.tile([C, SLICE], fp32, tag=f"r2f{o}")
                    nc.vector.tensor_tensor(
                        out=r2f[:], in0=r1f[:], in1=l1b[:], op=OP.subtract
                    )
                    nc.scalar.copy(s2t[C:, :], r2f[:])
                else:
                    s2t = None
                heads.append((s1t, s2t))

            # ================= big matmuls =================
            pg = ps.tile([SLICE, N2], fp32, tag="pg")
            p0 = ps.tile([SLICE, N2], fp32, tag="p0")
            p1 = ps.tile([SLICE, N2], fp32, tag="p1")
            for j in range(2):
                sl = slice(j * H, (j + 1) * H)
                nc.tensor.matmul(pg[:, sl], g_s1[:], R_hh[:, sl],
                                 start=True, stop=False)
                nc.tensor.matmul(pg[:, sl], g_s1[:], R_l1[:, sl],
                                 start=False, stop=False)
                nc.tensor.matmul(pg[:, sl], g_s2[:], R_mx[:, sl],
                                 start=False, stop=True)

            # transpose -> [24,128] psum -> sbuf -> cast reshape [3,1024] bf16
            ptr = ps.tile([24, 128], fp32, tag="rb")
            nc.tensor.transpose(ptr[:], T24[:], ident)
            r2t24 = sb.tile([24, 128], fp32)
            nc.vector.tensor_copy(r2t24[:], ptr[:])
            r2row3 = sb.tile([3, N2], bf16)
            nc.gpsimd.dma_start(r2row3[:], r2t24[:])  # SWDGE casts f32->bf16

            for j in range(2):
                sl = slice(j * H, (j + 1) * H)
                for o, pt in ((0, p0), (1, p1)):
                    s1t, s2t = heads[o]
                    if HEAD_3PASS:
                        nc.tensor.matmul(pt[:, sl], s1t[:], R_hh[:, sl],
                                         start=True, stop=False)
                        nc.tensor.matmul(pt[:, sl], s1t[:], R_l1[:, sl],
                                         start=False, stop=False)
                        nc.tensor.matmul(pt[:, sl], s2t[:], R_mx[:, sl],
                                         start=False, stop=True)
                    else:
                        nc.tensor.matmul(pt[:, sl], s1t[:], R_hh[:, sl],
                                         start=True, stop=False)
                        nc.tensor.matmul(pt[:, sl], s1t[:], R_l1[:, sl],
                                         start=False, stop=True)

            # r2 broadcast: ONE K=3 bf16 matmul per half sums h+l1+l2
            r2B = ps.tile([SLICE, N2], fp32, tag="rb")
            for j in range(2):
                sl = slice(j * H, (j + 1) * H)
                nc.tensor.matmul(r2B[:, sl], ones3[:], r2row3[:, sl])

            # ================= outputs (halves, pipelined) =========
            r2Bs = sb.tile([SLICE, N2], fp32)
            d2 = sb.tile([SLICE, N2], fp32)
            dist = sb.tile([SLICE, N2], fp32)
            outsb = sb.tile([SLICE, N2, 2], fp32)
            for j in range(2):
                sl = slice(j * H, (j + 1) * H)
                nc.scalar.copy(r2Bs[:, sl], r2B[:, sl])
                nc.vector.scalar_tensor_tensor(
                    out=d2[:, sl], in0=pg[:, sl], scalar=r1m2[:],
                    in1=r2Bs[:, sl], op0=OP.mult, op1=OP.mult,
                )
                nc.scalar.activation(
                    dist[:, sl], d2[:, sl], AF.Sqrt, bias=two[:], scale=1.0
                )
                nc.sync.dma_start(out_n[:, sl], dist[:, sl])
            for j in range(2):
                sl = slice(j * H, (j + 1) * H)
                nc.vector.tensor_scalar_add(outsb[:, sl, 0], p0[:, sl], bcol[:, 0:1])
                nc.scalar.activation(
                    outsb[:, sl, 1], p1[:, sl], AF.Identity,
                    bias=bcol[:, 1:2], scale=1.0,
                )
                nc.scalar.dma_start(out_o[:, sl, :], outsb[:, sl, :])

    nc.compile()
    return nc


def _get_program():
    if "nc" not in _CACHE:
        _CACHE["nc"] = _build_program()
    return _CACHE["nc"]


def make_in_maps(out1, out2, W, bias):
    import ml_dtypes

    bf = ml_dtypes.bfloat16
    v1 = np.ascontiguousarray(out1.reshape(C, N1), dtype=np.float32)
    v2 = np.ascontiguousarray(out2.reshape(C, N2), dtype=np.float32)
    W = np.asarray(W, dtype=np.float32)
    bias = np.asarray(bias, dtype=np.float32)

    x2h, x2l1, x2l2 = _split3_np(v2)
    rstk = np.ascontiguousarray(
        np.concatenate(
            [
                np.concatenate([x2h, x2h], axis=0),
                np.concatenate([x2l1, x2l1], axis=0),
                np.concatenate([x2l2, x2h], axis=0),
            ],
            axis=1,
        )
    )

    ident = np.eye(128, dtype=np.float32)
    v2t = v2.T.reshape(8, 128, C).transpose(1, 0, 2).reshape(128, 512)
    bcolfull = np.repeat(bias[None, :], 128, axis=0)

    in_maps = []
    for k in range(NCORES):
        x1 = np.ascontiguousarray(v1[:, k * SLICE : (k + 1) * SLICE])
        h, l1, l2 = _split3_np(x1)
        g1 = np.concatenate([h, l1], axis=0)  # [128, 128] bf16
        g2 = np.concatenate([h, l2], axis=0)
        f32e = np.ascontiguousarray(
            np.concatenate(
                [v2t, g1.view(np.float32), g2.view(np.float32)], axis=1
            )
        )
        f32part = np.concatenate([x1, W.T], axis=1)  # [64, 130] f32
        x1pk = np.ascontiguousarray(f32part.view(bf).reshape(C, -1))
        f32f = np.ascontiguousarray(
            np.concatenate([x1.T, ident, bcolfull], axis=1)
        )
        in_maps.append(
            {"f32e": f32e, "f32f": f32f, "x1pk": x1pk, "rstk": rstk}
        )
    return in_maps


def gather_results(results):
    out = np.concatenate(
        [results[k]["out_o"].reshape(SLICE * N2, 2) for k in range(NCORES)], axis=0
    )
    out_norm = np.concatenate([results[k]["out_n"] for k in range(NCORES)], axis=0)[
        None, :, :
    ]
    return out, out_norm


def kernel(out1, out2, W, bias):
    from concourse.bass_utils import run_bass_kernel_spmd

    nc = _get_program()
    in_maps = make_in_maps(
        np.asarray(out1), np.asarray(out2), np.asarray(W), np.asarray(bias)
    )
    res = run_bass_kernel_spmd(nc, in_maps, list(range(NCORES)))
    return gather_results(res.results)


# revision 12
# speedup vs baseline: 1.1118x; 1.0128x over previous
"""Trainium2 Bass kernel for the DescriptorMatcher all-pairs problem.

Reference semantics (v1 = out1 as [N1, C], v2 = out2 as [N2, C]):
  out[n1*N2+n2, o]   = sum_c v1[n1,c] * W[o,c] * v2[n2,c] + bias[o]
  out_norm[0,n1,n2]  = || v1/(eps+|v1|) - v2/(eps+|v2|) ||
                     = sqrt(2 - 2*r1[n1]*r2[n2]*G[n1,n2])  (+O(3e-7))
  with G = v1 @ v2.T and r = 1/(eps+|v|).

Sharding: N1 split across 8 cores (128 rows each), v2 replicated.

Device program highlights:
  - Big matmuls in bf16 with mantissa splits (h+l1+l2 covers f32)
    K-stacked in pairs; per [128,1024] output three K=128 bf16 passes:
    [h;l1]@[yh;yh] + [h;l1]@[yl1;yl1] + [h;l2]@[yl2;yh]  (err ~2^-26).
    The out-head optionally runs 2-pass (drops the l2 cross terms).
  - r1/r2 norm chains in column orientation (128 lanes) with one Newton
    step to fix the ACT Sqrt table error (~7e-6 -> ~1e-7).
  - r2 row: 3-way bf16 split in columns, one PE transpose [128,24] ->
    [24,128], SBUF->SBUF cast-DMA reshape to [3,1024] bf16, then ONE
    K=3 ones-matmul sums the components while broadcasting -> r2B.
  - dist = ACT_Sqrt((G_psum * -2r1) * r2B + 2.0).
"""

import numpy as np

C = 64
N1 = 1024
N2 = 1024
NCORES = 8
SLICE = N1 // NCORES  # 128
EPS = 1e-6
H = 512  # N-half
HEAD_3PASS = False  # True: exact-ish out head (3 bf16 passes per channel)

_CACHE = {}


def _split3_np(x):
    import ml_dtypes

    bf = ml_dtypes.bfloat16
    h = x.astype(bf)
    r1 = (x - h.astype(np.float32)).astype(np.float32)
    l1 = r1.astype(bf)
    l2 = (r1 - l1.astype(np.float32)).astype(bf)
    return h, l1, l2


def _build_program():
    import concourse.bacc as bacc
    import concourse.mybir as mybir
    import concourse.tile as tile
    from concourse._compat import get_trn_type

    fp32 = mybir.dt.float32
    bf16 = mybir.dt.bfloat16
    AF = mybir.ActivationFunctionType
    OP = mybir.AluOpType
    X = mybir.AxisListType.X

    nc = bacc.Bacc(get_trn_type() or "TRN2", target_bir_lowering=False, debug=False)

    # Trim the framework preamble: drop the const-AP memsets (unused; all
    # our activation biases are explicit APs) and the start all-engine
    # barrier. Body ordering is fully covered by Tile-generated semaphores,
    # and removing the barrier stops every engine from waiting ~7us for the
    # GPSIMD Q7 core to boot before even issuing input DMAs.
    _blk = nc.m.functions[0].blocks[0]
    _blk.instructions = [
        i for i in _blk.instructions
        if type(i).__name__ not in ("InstMemset", "InstDrain", "InstEventSemaphore")
    ]

    # ---- DRAM I/O (packed into 3 inputs)
    # f32e: x2t | g1-bitcast | g2-bitcast   [128, 512+64+64] f32
    # (g1 = [x1h;x1l1], g2 = [x1h;x1l2] bf16 [128,128] carried as f32 bytes)
    f32e = nc.dram_tensor("f32e", [SLICE, 512 + 128], fp32, kind="ExternalInput").ap()
    # f32f: x1t | identity | bcol   [128, 64+128+2] f32
    f32f = nc.dram_tensor(
        "f32f", [SLICE, C + 128 + 2], fp32, kind="ExternalInput"
    ).ap()
    # x1pk (bf16): x1-f32-bitcast | wc-bitcast   [64, 260]
    x1pk = nc.dram_tensor(
        "x1pk", [C, 2 * (SLICE + 2)], bf16, kind="ExternalInput"
    ).ap()
    # rstk: x2 K-stacked bf16 rhs: [x2h;x2h] | [x2l1;x2l1] | [x2l2;x2h]
    rstk = nc.dram_tensor("rstk", [2 * C, 3 * N2], bf16, kind="ExternalInput").ap()
    out_o = nc.dram_tensor("out_o", [SLICE, N2, 2], fp32, kind="ExternalOutput").ap()
    out_n = nc.dram_tensor("out_n", [SLICE, N2], fp32, kind="ExternalOutput").ap()

    with tile.TileContext(nc) as tc:
        with (
            tc.tile_pool(name="sb", bufs=1) as sb,
            tc.tile_pool(name="ps", bufs=1, space="PSUM") as ps,
        ):
            # ================= input DMAs =================
            tf32e = sb.tile([SLICE, 512 + 128], fp32)
            nc.sync.dma_start(tf32e[:], f32e)
            tx1pk = sb.tile([C, 2 * (SLICE + 2)], bf16)
            nc.sync.dma_start(tx1pk[:], x1pk)
            trstk = sb.tile([2 * C, 3 * N2], bf16)
            nc.scalar.dma_start(trstk[:, 0:N2], rstk[:, 0:N2])
            nc.scalar.dma_start(trstk[:, N2 : 2 * N2], rstk[:, N2 : 2 * N2])
            nc.scalar.dma_start(trstk[:, 2 * N2 :], rstk[:, 2 * N2 :])
            tf32f = sb.tile([SLICE, C + 128 + 2], fp32)
            nc.scalar.dma_start(tf32f[:], f32f)

            x2t = tf32e[:, 0:512]  # [128, 8*64] flat
            g_s1 = tf32e[:, 512:576].bitcast(bf16)  # [128, 128] [x1h;x1l1]
            g_s2 = tf32e[:, 576:640].bitcast(bf16)  # [128, 128] [x1h;x1l2]
            x1t = tf32f[:, 0:C]
            ident = tf32f[:, C : C + 128]
            bcol = tf32f[:, C + 128 : C + 128 + 2]

            # constants + ACT table prefetch (Identity) while DMAs land
            two = sb.tile([SLICE, 1], fp32)
            nc.vector.memset(two[:], 2.0)
            zerob = sb.tile([SLICE, 1], fp32)
            nc.vector.memset(zerob[:], 0.0)
            dummy = sb.tile([SLICE, 1], fp32)
            nc.scalar.activation(dummy[:], two[:], AF.Identity, bias=zerob[:],
                                 scale=1.0)
            x1f32 = tx1pk[:, 0 : 2 * (SLICE + 2)].bitcast(fp32)  # [64, 130]
            x1f = x1f32[:, 0:SLICE]
            wcs = x1f32[:, SLICE : SLICE + 2]
            R_hh = trstk[:, 0:N2]
            R_l1 = trstk[:, N2 : 2 * N2]
            R_mx = trstk[:, 2 * N2 : 3 * N2]

            # ================= r2 chain (col, [128,8]) =================
            x2sq = sb.tile([SLICE, 512], fp32)
            nc.vector.tensor_mul(x2sq[:], x2t, x2t)
            s2c = sb.tile([SLICE, 8], fp32)
            nc.vector.reduce_sum(
                s2c[:], x2sq[:].rearrange("p (t c) -> p t c", c=C), axis=X
            )
            # ah = 0.5*sqrt(s2) via sqrt(0.25*s2); n = s2/(4*ah) + ah
            ah2 = sb.tile([SLICE, 8], fp32)
            nc.scalar.activation(ah2[:], s2c[:], AF.Sqrt, bias=zerob[:], scale=0.25)
            i2 = sb.tile([SLICE, 8], fp32)
            nc.vector.reciprocal(i2[:], ah2[:])
            t2 = sb.tile([SLICE, 8], fp32)
            nc.vector.tensor_mul(t2[:], s2c[:], i2[:])
            n2c = sb.tile([SLICE, 8], fp32)
            nc.vector.scalar_tensor_tensor(
                out=n2c[:], in0=t2[:], scalar=0.25, in1=ah2[:],
                op0=OP.mult, op1=OP.add,
            )
            r2c8 = sb.tile([SLICE, 8], fp32)
            nc.vector.reciprocal(r2c8[:], n2c[:])
            # 3-way bf16 split (values held in f32), packed [128, 24]
            T24 = sb.tile([SLICE, 24], fp32)
            bsc = sb.tile([SLICE, 8], bf16)
            nc.scalar.copy(bsc[:], r2c8[:])
            nc.scalar.copy(T24[:, 0:8], bsc[:])
            res1 = sb.tile([SLICE, 8], fp32)
            nc.vector.tensor_tensor(
                out=res1[:], in0=r2c8[:], in1=T24[:, 0:8], op=OP.subtract
            )
            b1c = sb.tile([SLICE, 8], bf16)
            nc.scalar.copy(b1c[:], res1[:])
            nc.scalar.copy(T24[:, 8:16], b1c[:])
            nc.vector.tensor_tensor(
                out=T24[:, 16:24], in0=res1[:], in1=T24[:, 8:16], op=OP.subtract
            )
            ones3 = sb.tile([3, SLICE], bf16)
            nc.vector.memset(ones3[:], 1.0)

            # ================= r1 chain (col, [128,1]) =================
            x1sq = sb.tile([SLICE, C], fp32)
            nc.vector.tensor_mul(x1sq[:], x1t, x1t)
            s1c = sb.tile([SLICE, 1], fp32)
            nc.vector.reduce_sum(s1c[:], x1sq[:], axis=X)
            a1 = sb.tile([SLICE, 1], fp32)
            nc.scalar.activation(a1[:], s1c[:], AF.Sqrt, bias=zerob[:], scale=1.0)
            i1 = sb.tile([SLICE, 1], fp32)
            nc.vector.reciprocal(i1[:], a1[:])
            t1 = sb.tile([SLICE, 1], fp32)
            nc.vector.tensor_mul(t1[:], s1c[:], i1[:])
            u1 = sb.tile([SLICE, 1], fp32)
            nc.vector.tensor_add(u1[:], a1[:], t1[:])
            n1c = sb.tile([SLICE, 1], fp32)
            nc.vector.tensor_scalar(
                out=n1c[:], in0=u1[:], scalar1=0.5, scalar2=EPS,
                op0=OP.mult, op1=OP.add,
            )
            r1c = sb.tile([SLICE, 1], fp32)
            nc.vector.reciprocal(r1c[:], n1c[:])
            r1m2 = sb.tile([SLICE, 1], fp32)
            nc.vector.tensor_scalar_mul(r1m2[:], r1c[:], -2.0)

            # ================= lhsT preps (out head) =================
            heads = []
            for o in range(2):
                lf = sb.tile([C, SLICE], fp32, tag=f"lf{o}")
                nc.vector.tensor_scalar_mul(lf[:], x1f, wcs[:, o : o + 1])
                s1t = sb.tile([2 * C, SLICE], bf16, tag=f"hs1{o}")  # [h; l1]
                nc.scalar.copy(s1t[0:C, :], lf[:])
                l1b = sb.tile([C, SLICE], bf16, tag=f"l1b{o}")
                r1f = sb.tile([C, SLICE], fp32, tag=f"r1f{o}")
                nc.vector.tensor_tensor(
                    out=r1f[:], in0=lf[:], in1=s1t[0:C, :], op=OP.subtract
                )
                nc.scalar.copy(l1b[:], r1f[:])
                nc.scalar.copy(s1t[C:, :], l1b[:])
                if HEAD_3PASS:
                    s2t = sb.tile([2 * C, SLICE], bf16, tag=f"hs2{o}")  # [h; l2]
                    nc.scalar.copy(s2t[0:C, :], s1t[0:C, :])
                    r2f = sb.tile([C, SLICE], fp32, tag=f"r2f{o}")
                    nc.vector.tensor_tensor(
                        out=r2f[:], in0=r1f[:], in1=l1b[:], op=OP.subtract
                    )
                    nc.scalar.copy(s2t[C:, :], r2f[:])
                else:
                    s2t = None
                heads.append((s1t, s2t))

            # ================= big matmuls =================
            pg = ps.tile([SLICE, N2], fp32, tag="pg")
            p0 = ps.tile([SLICE, N2], fp32, tag="p0")
            p1 = ps.tile([SLICE, N2], fp32, tag="p1")
            for j in range(2):
                sl = slice(j * H, (j + 1) * H)
                nc.tensor.matmul(pg[:, sl], g_s1[:], R_hh[:, sl],
                                 start=True, stop=False)
                nc.tensor.matmul(pg[:, sl], g_s1[:], R_l1[:, sl],
                                 start=False, stop=False)
                nc.tensor.matmul(pg[:, sl], g_s2[:], R_mx[:, sl],
                                 start=False, stop=True)

            # transpose -> [24,128] psum -> sbuf -> cast reshape [3,1024] bf16
            ptr = ps.tile([24, 128], fp32, tag="rb")
            nc.tensor.transpose(ptr[:], T24[:], ident)
            r2t24 = sb.tile([24, 128], fp32)
            nc.vector.tensor_copy(r2t24[:], ptr[:])
            r2row3 = sb.tile([3, N2], bf16)
            nc.gpsimd.dma_start(r2row3[:], r2t24[:])  # SWDGE casts f32->bf16

            r2B = ps.tile([SLICE, N2], fp32, tag="rb")

            def head_mms(j):
                sl = slice(j * H, (j + 1) * H)
                for o, pt in ((0, p0), (1, p1)):
                    s1t, s2t = heads[o]
                    nc.tensor.matmul(pt[:, sl], s1t[:], R_hh[:, sl],
                                     start=True, stop=False)
                    if HEAD_3PASS:
                        nc.tensor.matmul(pt[:, sl], s1t[:], R_l1[:, sl],
                                         start=False, stop=False)
                        nc.tensor.matmul(pt[:, sl], s2t[:], R_mx[:, sl],
                                         start=False, stop=True)
                    else:
                        nc.tensor.matmul(pt[:, sl], s1t[:], R_l1[:, sl],
                                         start=False, stop=True)

            head_mms(0)
            # r2 broadcast: ONE K=3 bf16 matmul per half sums h+l1+l2
            for j in range(2):
                sl = slice(j * H, (j + 1) * H)
                nc.tensor.matmul(r2B[:, sl], ones3[:], r2row3[:, sl])
            head_mms(1)

            # ================= outputs =========
            # pg -> sbuf early (ACT), then per-quarter stt reads r2B psum
            pgs = sb.tile([SLICE, N2], fp32)
            for j in range(2):
                sl = slice(j * H, (j + 1) * H)
                nc.scalar.copy(pgs[:, sl], pg[:, sl])
            d2 = sb.tile([SLICE, N2], fp32)
            dist = sb.tile([SLICE, N2], fp32)
            outsb = sb.tile([SLICE, N2, 2], fp32)
            Q = 256
            for q in range(4):
                sl = slice(q * Q, (q + 1) * Q)
                nc.vector.scalar_tensor_tensor(
                    out=d2[:, sl], in0=r2B[:, sl], scalar=r1m2[:],
                    in1=pgs[:, sl], op0=OP.mult, op1=OP.mult,
                )
                nc.scalar.activation(
                    dist[:, sl], d2[:, sl], AF.Sqrt, bias=two[:], scale=1.0
                )
            for j in range(2):
                sl = slice(j * H, (j + 1) * H)
                nc.sync.dma_start(out_n[:, sl], dist[:, sl])
            for j in range(2):
                sl = slice(j * H, (j + 1) * H)
                nc.vector.tensor_scalar_add(outsb[:, sl, 0], p0[:, sl], bcol[:, 0:1])
                nc.scalar.activation(
                    outsb[:, sl, 1], p1[:, sl], AF.Identity,
                    bias=bcol[:, 1:2], scale=1.0,
                )
                nc.scalar.dma_start(out_o[:, sl, :], outsb[:, sl, :])

    nc.compile()
    return nc


def _get_program():
    if "nc" not in _CACHE:
        _CACHE["nc"] = _build_program()
    return _CACHE["nc"]


def make_in_maps(out1, out2, W, bias):
    import ml_dtypes

    bf = ml_dtypes.bfloat16
    v1 = np.ascontiguousarray(out1.reshape(C, N1), dtype=np.float32)
    v2 = np.ascontiguousarray(out2.reshape(C, N2), dtype=np.float32)
    W = np.asarray(W, dtype=np.float32)
    bias = np.asarray(bias, dtype=np.float32)

    x2h, x2l1, x2l2 = _split3_np(v2)
    rstk = np.ascontiguousarray(
        np.concatenate(
            [
                np.concatenate([x2h, x2h], axis=0),
                np.concatenate([x2l1, x2l1], axis=0),
                np.concatenate([x2l2, x2h], axis=0),
            ],
            axis=1,
        )
    )

    ident = np.eye(128, dtype=np.float32)
    v2t = v2.T.reshape(8, 128, C).transpose(1, 0, 2).reshape(128, 512)
    bcolfull = np.repeat(bias[None, :], 128, axis=0)

    in_maps = []
    for k in range(NCORES):
        x1 = np.ascontiguousarray(v1[:, k * SLICE : (k + 1) * SLICE])
        h, l1, l2 = _split3_np(x1)
        g1 = np.concatenate([h, l1], axis=0)  # [128, 128] bf16
        g2 = np.concatenate([h, l2], axis=0)
        f32e = np.ascontiguousarray(
            np.concatenate(
                [v2t, g1.view(np.float32), g2.view(np.float32)], axis=1
            )
        )
        f32part = np.concatenate([x1, W.T], axis=1)  # [64, 130] f32
        x1pk = np.ascontiguousarray(f32part.view(bf).reshape(C, -1))
        f32f = np.ascontiguousarray(
            np.concatenate([x1.T, ident, bcolfull], axis=1)
        )
        in_maps.append(
            {"f32e": f32e, "f32f": f32f, "x1pk": x1pk, "rstk": rstk}
        )
    return in_maps


def gather_results(results):
    out = np.concatenate(
        [results[k]["out_o"].reshape(SLICE * N2, 2) for k in range(NCORES)], axis=0
    )
    out_norm = np.concatenate([results[k]["out_n"] for k in range(NCORES)], axis=0)[
        None, :, :
    ]
    return out, out_norm


def kernel(out1, out2, W, bias):
    from concourse.bass_utils import run_bass_kernel_spmd

    nc = _get_program()
    in_maps = make_in_maps(
        np.asarray(out1), np.asarray(out2), np.asarray(W), np.asarray(bias)
    )
    res = run_bass_kernel_spmd(nc, in_maps, list(range(NCORES)))
    return gather_results(res.results)
